# revision 2
# baseline (speedup 1.0000x reference)
"""AGNNConv Trainium2 kernel v3: sharded table upload + on-device allgather
+ indirect gathers + batched per-tile compute.

Wall time through the axon tunnel is RTT (~80ms) + input upload (~10.7GB/s)
+ device time. v3 minimizes input bytes: each core uploads only a 1/8 shard
of the bf16 node table [rnorm | H | norm] (~1.6MB) plus its edge offsets and
labels (~0.7MB); the full table is assembled on-device with one AllGather.
Per-edge rows are then gathered on-device (indirect DMA) and the per-tile
compute is batched (one DVE/ACT op per stage covering all k chunks).

Math (identical to the reference up to fp):
  cos_e = <H[src], H[dst]>,  w_e = exp(beta * cos_e)
  out[n] = sum_{dst_e=n} w_e * feat[src_e] / sum_{dst_e=n} w_e
  (softmax max-shift cancels in the ratio; |beta*cos| <= |beta|)

rhs trick (from baseline): gathered src row is [rnorm | H | norm]; scaling
[rnorm | H] by alpha = w*norm yields [w | feat*w], so one matmul per chunk
accumulates both the denominator and the weighted messages.
"""

import math
import os

import numpy as np

import concourse.bass as bass
import concourse.mybir as mybir
from concourse.bass import IndirectOffsetOnAxis
from concourse.tile import TileContext

P = 128
D = 128
N_CORES = 8
TC = 2 + D  # table cols: [rnorm | H | norm]
RC = 1 + D  # rhs cols / psum cols: [w | feat*w]

F32 = mybir.dt.float32
BF16 = mybir.dt.bfloat16
I32 = mybir.dt.int32
NP_BF16 = mybir.dt.np(BF16)

AF = mybir.ActivationFunctionType
ALU = mybir.AluOpType


def _legalize_waits(nc):
    """Walrus allows at most one embedded semaphore wait per standard engine
    instruction and none on raw-ISA ones; spill extras into standalone
    EventSemaphore waits on the same engine (identical semantics)."""
    import bass_rust

    dummy = nc.alloc_semaphore(name="legwait-dummy")
    ctr = [0]
    for f in nc.m.functions:
        for bb in f.blocks:
            lst = bb.instructions
            out = []
            changed = False
            for inst in lst:
                si = inst.sync_info
                tname = type(inst).__name__
                if tname == "InstEventSemaphore":
                    out.append(inst)
                    continue
                ok_one = tname in (
                    "InstTensorTensor",
                    "InstActivation",
                    "InstMatmult",
                    "InstLdweights",
                    "InstTensorCopy",
                    "InstTensorScalarPtr",
                    "InstReciprocal",
                    "InstMemset",
                    "InstTensorReduce",
                    "InstDMACopy",
                    "InstDrain",
                    "InstIota",
                    "InstTensorScalarAffineSelect",
                )
                lim = 1 if ok_one else 0
                if si is not None and si.on_wait and len(si.on_wait) > lim:
                    waits = list(si.on_wait)
                    spill = waits[: len(waits) - lim]
                    for w in spill:
                        ev = mybir.InstEventSemaphore(
                            name=f"legwait-{ctr[0]}", ins=[], outs=[]
                        )
                        ctr[0] += 1
                        ev.engine = inst.engine
                        u = bass_rust.SyncUpdate(
                            sync_type="semaphore",
                            id=dummy.num,
                            ant_name="legwait-dummy",
                            update_mode="sem-inc",
                            update_value=1,
                        )
                        ev.sync_info = mybir.SyncInfo(on_wait=[w], on_update=[u])
                        out.append(ev)
                    si.on_wait = waits[len(waits) - lim :]
                    changed = True
                out.append(inst)
            if changed:
                bb.instructions = out


def build_graph(n_pos, k_list, dst_slab=True, legalize=True):
    """SPMD graph: n_pos owned dst tiles, k_list[t] 128-edge chunks each.
    dst_slab=True selects dst rows from the owned tile's resident slab via a
    transposed one-hot matmul instead of per-chunk indirect gathers (halves
    the descriptor-latency-bound gather traffic)."""
    sumk = sum(k_list)
    kmax = max(k_list)
    rows_core = n_pos * P
    rows_full = N_CORES * rows_core
    nc = bass.Bass(num_devices=N_CORES)

    tshard_ext = nc.declare_dram_parameter(
        "tshard", [rows_core * TC], BF16, isOutput=False
    )
    soff_ext = nc.declare_dram_parameter("soff", [P, sumk], I32, isOutput=False)
    if dst_slab:
        slaboff_ext = nc.declare_dram_parameter(
            "slaboff", [P, n_pos], I32, isOutput=False
        )
        dstlf_ext = nc.declare_dram_parameter(
            "dstlf", [1, sumk * P], BF16, isOutput=False
        )
        iotac_ext = nc.declare_dram_parameter("iotac", [P, 1], BF16, isOutput=False)
    else:
        doff_ext = nc.declare_dram_parameter("doff", [P, sumk], I32, isOutput=False)
    dstl_ext = nc.declare_dram_parameter("dstl", [P, sumk], BF16, isOutput=False)
    beta_ext = nc.declare_dram_parameter("beta", [1, 1], F32, isOutput=False)
    iota_ext = nc.declare_dram_parameter("iota", [P, P], BF16, isOutput=False)
    out_ext = nc.declare_dram_parameter("out", [n_pos * P, D], F32, isOutput=True)

    tbl_full = nc.dram_tensor("tbl_full", [rows_full * TC], BF16)
    tbl_rows = tbl_full[:].rearrange("(r c) -> r c", c=1)
    # collectives may not read IO tensors; stage the shard in internal DRAM
    tbl_mine = nc.dram_tensor("tbl_mine", [rows_core * TC], BF16)

    with TileContext(nc) as tc:
        with (
            tc.tile_pool(name="const", bufs=1) as constp,
            tc.tile_pool(name="mega", bufs=4) as mega,
            tc.tile_pool(name="work", bufs=3) as work,
            tc.tile_pool(name="small", bufs=6) as small,
            tc.tile_pool(name="psum", bufs=4, space="PSUM") as psum,
            tc.tile_pool(name="psum2", bufs=4, space="PSUM") as psum2,
        ):
            nc.sync.dma_start(out=tbl_mine[:], in_=tshard_ext[:])
            nc.gpsimd.collective_compute(
                kind="AllGather",
                op=ALU.bypass,
                replica_groups=[list(range(N_CORES))],
                ins=[tbl_mine[:]],
                outs=[tbl_full[:]],
            )

            iota_t = constp.tile([P, P], BF16)
            nc.sync.dma_start(out=iota_t[:], in_=iota_ext[:, :])
            beta_t = constp.tile([P, 1], F32)
            nc.sync.dma_start(out=beta_t[:], in_=beta_ext[:, :].to_broadcast((P, 1)))
            dstl_all = constp.tile([P, sumk], BF16)
            nc.sync.dma_start(out=dstl_all[:], in_=dstl_ext[:, :])
            soff_all = constp.tile([P, sumk], I32)
            nc.sync.dma_start(out=soff_all[:], in_=soff_ext[:, :])
            if dst_slab:
                slaboff_all = constp.tile([P, n_pos], I32)
                nc.sync.dma_start(out=slaboff_all[:], in_=slaboff_ext[:, :])
                iota_c = constp.tile([P, 1], BF16)
                nc.sync.dma_start(out=iota_c[:], in_=iotac_ext[:, :])
            else:
                doff_all = constp.tile([P, sumk], I32)
                nc.sync.dma_start(out=doff_all[:], in_=doff_ext[:, :])

            off = 0
            for t in range(n_pos):
                k = k_list[t]
                hs = mega.tile([P, kmax * TC], BF16, tag="hs")
                hd = mega.tile([P, kmax * D], BF16, tag="hd")
                # NOTE: indirect-DMA out APs must stay 2D — the HW ucode
                # mishandles a 3D destination (sim accepts it); one gather
                # per 128-edge chunk, writing a 2D slice of the mega tile.
                for j0 in range(k):
                    nc.gpsimd.indirect_dma_start(
                        out=hs[:, j0 * TC : (j0 + 1) * TC],
                        out_offset=None,
                        in_=tbl_rows,
                        in_offset=IndirectOffsetOnAxis(
                            ap=soff_all[:, off + j0 : off + j0 + 1], axis=0
                        ),
                    )
                if dst_slab:
                    # resident H slab for the owned dst tile (per-core rows
                    # via data-driven offsets), selected per edge with a
                    # transposed one-hot matmul
                    slab = mega.tile([P, D], BF16, tag="slab")
                    nc.gpsimd.indirect_dma_start(
                        out=slab[:],
                        out_offset=None,
                        in_=tbl_rows,
                        in_offset=IndirectOffsetOnAxis(
                            ap=slaboff_all[:, t : t + 1], axis=0
                        ),
                    )
                    dstl_rep = work.tile([P, kmax * P], BF16, tag="dstl_rep")
                    nc.sync.dma_start(
                        out=dstl_rep[:, : k * P],
                        in_=dstlf_ext[0:1, off * P : (off + k) * P].to_broadcast(
                            (P, k * P)
                        ),
                    )
                    sohT = work.tile([P, kmax * P], BF16, tag="sohT")
                    st3 = sohT[:, : k * P].rearrange("p (k c) -> p k c", c=P)
                    nc.vector.tensor_tensor(
                        out=st3,
                        in0=dstl_rep[:, : k * P].rearrange("p (k c) -> p k c", c=P),
                        in1=iota_c[:].unsqueeze(2).broadcast_to((P, k, P)),
                        op=ALU.is_equal,
                    )
                    for j0 in range(k):
                        hdp = psum2.tile([P, D], F32, tag="hdp")
                        nc.tensor.matmul(
                            out=hdp[:],
                            lhsT=sohT[:, j0 * P : (j0 + 1) * P],
                            rhs=slab[:],
                            start=True,
                            stop=True,
                        )
                        nc.scalar.activation(
                            hd[:, j0 * D : (j0 + 1) * D], hdp[:], AF.Copy
                        )
                else:
                    for j0 in range(k):
                        nc.gpsimd.indirect_dma_start(
                            out=hd[:, j0 * D : (j0 + 1) * D],
                            out_offset=None,
                            in_=tbl_rows,
                            in_offset=IndirectOffsetOnAxis(
                                ap=doff_all[:, off + j0 : off + j0 + 1], axis=0
                            ),
                            element_offset=1,
                        )
                hs3 = hs[:, : k * TC].rearrange("p (k c) -> p k c", c=TC)
                hd3 = hd[:, : k * D].rearrange("p (k c) -> p k c", c=D)

                prod = work.tile([P, kmax * D], BF16, tag="prod")
                p3 = prod[:, : k * D].rearrange("p (k c) -> p k c", c=D)
                nc.vector.tensor_tensor(
                    out=p3, in0=hs3[:, :, 1 : 1 + D], in1=hd3, op=ALU.mult
                )
                dotp = small.tile([P, kmax], F32, tag="dotp")
                nc.vector.reduce_sum(
                    dotp[:, :k].unsqueeze(2), p3, axis=mybir.AxisListType.X
                )
                w = small.tile([P, kmax], F32, tag="w")
                nc.scalar.activation(w[:, :k], dotp[:, :k], AF.Exp, scale=beta_t[:])
                ns = small.tile([P, kmax], F32, tag="ns")
                nc.vector.tensor_copy(
                    out=ns[:, :k].unsqueeze(2), in_=hs3[:, :, 1 + D : 2 + D]
                )
                alpha = small.tile([P, kmax], F32, tag="alpha")
                nc.vector.tensor_tensor(
                    out=alpha[:, :k], in0=w[:, :k], in1=ns[:, :k], op=ALU.mult
                )
                ab = small.tile([P, kmax], BF16, tag="ab")
                nc.vector.tensor_copy(out=ab[:, :k], in_=alpha[:, :k])

                rhs = work.tile([P, kmax * RC], BF16, tag="rhs")
                r3 = rhs[:, : k * RC].rearrange("p (k c) -> p k c", c=RC)
                nc.vector.tensor_tensor(
                    out=r3,
                    in0=hs3[:, :, 0:RC],
                    in1=ab[:, :k].unsqueeze(2).broadcast_to((P, k, RC)),
                    op=ALU.mult,
                )

                soh = work.tile([P, kmax * P], BF16, tag="soh")
                s3 = soh[:, : k * P].rearrange("p (k c) -> p k c", c=P)
                nc.vector.tensor_tensor(
                    out=s3,
                    in0=dstl_all[:, off : off + k].unsqueeze(2).broadcast_to((P, k, P)),
                    in1=iota_t[:].unsqueeze(1).broadcast_to((P, k, P)),
                    op=ALU.is_equal,
                )

                pt = psum.tile([P, RC], F32)
                for j in range(k):
                    nc.tensor.matmul(
                        out=pt[:],
                        lhsT=soh[:, j * P : (j + 1) * P],
                        rhs=rhs[:, j * RC : (j + 1) * RC],
                        start=(j == 0),
                        stop=(j == k - 1),
                    )

                dmax = small.tile([P, 1], F32, tag="dmax")
                nc.vector.tensor_scalar_max(dmax[:], pt[:, 0:1], 1e-30)
                rec = small.tile([P, 1], F32, tag="rec")
                nc.vector.reciprocal(rec[:], dmax[:])
                ot = work.tile([P, D], F32, tag="ot")
                nc.vector.tensor_scalar_mul(ot[:], pt[:, 1 : 1 + D], rec[:])
                nc.sync.dma_start(out=out_ext[t * P : (t + 1) * P, :], in_=ot[:])
                off += k

    if legalize:
        _legalize_waits(nc)
    return nc


def shard_edges(feat, src, dst, n_nodes, n_cores):
    """Host prep: table shards (node-major rows [rnorm|H|norm] bf16) and
    per-core edge offset/label arrays in chunk order."""
    nt = math.ceil(n_nodes / P)
    n_pos = math.ceil(nt / n_cores)
    rows_core = n_pos * P
    rows_full = n_cores * rows_core

    g = dst // P
    order = np.argsort(g, kind="stable")
    g_sorted = g[order]
    starts = np.searchsorted(g_sorted, np.arange(nt + 1))

    counts = np.zeros((n_cores, n_pos), dtype=np.int64)
    for gg in range(nt):
        counts[gg % n_cores, gg // n_cores] = starts[gg + 1] - starts[gg]
    k_list = [max(1, int(math.ceil(counts[:, t].max() / P))) for t in range(n_pos)]
    sumk = sum(k_list)

    norm = np.maximum(np.linalg.norm(feat, axis=1), 1e-12).astype(np.float32)
    rnorm = (1.0 / norm).astype(np.float32)
    table = np.zeros((rows_full, TC), dtype=NP_BF16)
    table[:n_nodes, 0] = rnorm[:n_nodes].astype(NP_BF16)
    table[:n_nodes, 1 : 1 + D] = (feat * rnorm[:, None]).astype(NP_BF16)
    table[:n_nodes, 1 + D] = norm[:n_nodes].astype(NP_BF16)

    per_core = []
    for c in range(n_cores):
        src_pad = np.zeros(sumk * P, dtype=np.int64)
        dst_pad = np.zeros(sumk * P, dtype=np.int64)
        lbl_pad = np.full(sumk * P, -1.0, dtype=np.float32)
        col = 0
        for t in range(n_pos):
            k = k_list[t]
            gg = t * n_cores + c
            if gg < nt:
                e = order[starts[gg] : starts[gg + 1]]
                cnt = len(e)
                a = col * P
                src_pad[a : a + cnt] = src[e]
                dst_pad[a : a + cnt] = dst[e]
                lbl_pad[a : a + cnt] = (dst[e] - gg * P).astype(np.float32)
            col += k
        soff = np.ascontiguousarray(
            (src_pad * TC).astype(np.int32).reshape(sumk, P).T
        )
        dstl = np.ascontiguousarray(lbl_pad.astype(NP_BF16).reshape(sumk, P).T)
        # slab offsets: partition d of tile-pos t reads the H row of node
        # gg*128+d (element offset includes the +1 rnorm-col skip)
        gg_vec = np.arange(n_pos) * n_cores + c
        gg_vec = np.where(gg_vec < nt, gg_vec, 0)
        slaboff = (
            (gg_vec[None, :] * P + np.arange(P)[:, None]) * TC + 1
        ).astype(np.int32)
        per_core.append(
            {
                "tshard": table[c * rows_core : (c + 1) * rows_core].reshape(-1),
                "soff": soff,
                "dstl": dstl,
                "slaboff": np.ascontiguousarray(slaboff),
                "dstlf": lbl_pad.astype(NP_BF16).reshape(1, -1),
            }
        )
    return n_pos, k_list, per_core


def _run_pjrt_timed(nc, in_maps, n_cores, time_iters=0):
    import time

    import jax

    from concourse import bass2jax
    from concourse import mybir as mb
    from jax.sharding import Mesh, PartitionSpec
    from jax.experimental.shard_map import shard_map

    bass2jax.install_neuronx_cc_hook()

    part_name = nc.partition_id_tensor.name if nc.partition_id_tensor else None
    in_names, out_names, out_avals = [], [], []
    for alloc in nc.m.functions[0].allocations:
        if not isinstance(alloc, mb.MemoryLocationSet):
            continue
        name = alloc.memorylocations[0].name
        if alloc.kind == "ExternalInput":
            if name != part_name:
                in_names.append(name)
        elif alloc.kind == "ExternalOutput":
            out_names.append(name)
            out_avals.append(
                jax.core.ShapedArray(tuple(alloc.tensor_shape), mb.dt.np(alloc.dtype))
            )
    n_params = len(in_names)
    all_names = in_names + out_names
    if part_name is not None:
        all_names = all_names + [part_name]

    def _body(*args):
        operands = list(args)
        if part_name is not None:
            operands.append(bass2jax.partition_id_tensor())
        outs = bass2jax._bass_exec_p.bind(
            *operands,
            out_avals=tuple(out_avals),
            in_names=tuple(all_names),
            out_names=tuple(out_names),
            lowering_input_output_aliases=(),
            sim_require_finite=True,
            sim_require_nnan=True,
            nc=nc,
        )
        return tuple(outs)

    devices = jax.devices()[:n_cores]
    mesh = Mesh(np.asarray(devices), ("core",))
    sharded = jax.jit(
        shard_map(
            _body,
            mesh=mesh,
            in_specs=(PartitionSpec("core"),) * (n_params + len(out_names)),
            out_specs=(PartitionSpec("core"),) * len(out_names),
            check_rep=False,
        ),
        keep_unused=True,
    )
    concat_in = [
        jax.device_put(
            np.concatenate([np.asarray(in_maps[c][k]) for c in range(n_cores)], axis=0)
        )
        for k in in_names
    ] + [
        jax.device_put(np.zeros((n_cores * a.shape[0], *a.shape[1:]), a.dtype))
        for a in out_avals
    ]
    out_arrs = [np.asarray(o) for o in sharded(*concat_in)]

    if time_iters > 0:
        times = []
        for _ in range(time_iters):
            t0 = time.perf_counter()
            r = sharded(*concat_in)
            jax.block_until_ready(r)
            times.append(time.perf_counter() - t0)
        best = min(times)
        print(f"HW exec time: {best * 1e9:.0f} ns")
        print(f"wall times: {[f'{t*1e3:.2f}ms' for t in times]}")

    return [
        {
            name: out_arrs[i].reshape(n_cores, *out_avals[i].shape)[c]
            for i, name in enumerate(out_names)
        }
        for c in range(n_cores)
    ]


def kernel(feat, beta, src, dst):
    feat = np.asarray(feat, dtype=np.float32)
    beta = np.asarray(beta, dtype=np.float32)
    src = np.asarray(src, dtype=np.int64)
    dst = np.asarray(dst, dtype=np.int64)
    n_nodes = feat.shape[0]

    n_pos, k_list, per_core = shard_edges(feat, src, dst, n_nodes, N_CORES)
    nc = build_graph(n_pos, k_list)

    iota = np.broadcast_to(np.arange(P, dtype=np.float32), (P, P)).astype(NP_BF16)
    iotac = np.arange(P, dtype=np.float32).reshape(P, 1).astype(NP_BF16)
    in_maps = []
    for c in range(N_CORES):
        in_maps.append(
            {
                "beta": beta.reshape(1, 1).astype(np.float32),
                "iota": iota,
                "iotac": iotac,
                **per_core[c],
            }
        )

    iters = 5 if int(os.environ.get("BASS_KERNEL_TRACE", "0")) else 0
    results = _run_pjrt_timed(nc, in_maps, N_CORES, time_iters=iters)

    nt = math.ceil(n_nodes / P)
    out = np.zeros((nt * P, D), dtype=np.float32)
    for c in range(N_CORES):
        o = np.asarray(results[c]["out"])
        for t in range(n_pos):
            gg = t * N_CORES + c
            if gg < nt:
                out[gg * P : (gg + 1) * P] = o[t * P : (t + 1) * P]
    return out[:n_nodes]
